# revision 6
# baseline (speedup 1.0000x reference)
"""HGNN conv distributed Bass kernel for 8 TRN2 NeuronCores.

Computes  out = 0.5 * D_e ⊙ (MT.T @ (D_v ⊙ (MT @ (x @ W))))
with N=16384 nodes, E=8192 hyperedges, IN_FT=256, OUT_FT=128.

Sharding (node/data parallel per hint): MT columns, x rows and D_e are
sharded over nodes across the 8 cores; W and D_v are replicated. The
MT @ y contraction over nodes becomes a partial sum + AllReduce; the
MT.T @ z contraction over edges is local per node shard.

Per core the MT shard [E, N/8] streams through SBUF once (bf16,
host-cast) in 8 superblocks of EB=1024 edges. Phase 1 produces eyT
partials via PE-transposes of MT tiles; partials are AllReduced in 5
uneven groups of superblocks [1,2,2,2,1] so the first collective
starts early and the last leaves only one superblock of phase-2 work
after it — the serialized collective channel (~130us) hides under the
~220us of PE work. Phase 2 re-uses the natural MT tiles from SBUF with
the reduced z as stationary, accumulating nyT across superblocks.
x is loaded pre-transposed via the DMA XBAR so step A needs no PE
transposes.
"""

import functools
from contextlib import ExitStack

import ml_dtypes
import numpy as np

import concourse.bass as bass
import concourse.mybir as mybir
import concourse.tile as tile
from concourse import bacc
from concourse.bass_utils import run_bass_kernel_spmd
from concourse.masks import make_identity

P = 128
BF16 = mybir.dt.bfloat16
F32 = mybir.dt.float32

FULL_CFG = dict(N=16384, E=8192, IN=256, F=128, CORES=8, G=8)

# Superblocks per AllReduce call; uneven so the collective chain starts
# early and drains before the phase-2 tail.
AR_GROUPS = ((0,), (1, 2), (3, 4), (5, 6), (7,))


def _ceil_div(a, b):
    return -(-a // b)


def build_kernel(nc, cfg):
    N, E, IN, F, CORES, G = (
        cfg["N"], cfg["E"], cfg["IN"], cfg["F"], cfg["CORES"], cfg["G"])
    NS = N // CORES          # nodes per core
    EB = E // G              # edges per superblock
    ET = EB // P             # 128-edge chunks per superblock
    NJ = NS // P             # 128-node chunks (phase-1 contraction)
    KI = IN // P             # 128-in_ft chunks
    EH = _ceil_div(EB, 512)  # 512-edge groups per superblock (phase-1 psum)
    NQ = _ceil_div(NS, 512)  # 512-node groups (phase-2 free dim)
    EW = min(EB, 512)        # phase-1 psum group width
    NW = min(NS, 512)        # phase-2 moving free width
    assert EB % P == 0 and NS % P == 0 and IN % P == 0 and F == P

    mt = nc.dram_tensor("mt", [E, NS], BF16, kind="ExternalInput").ap()
    xs = nc.dram_tensor("xs", [NS, IN], BF16, kind="ExternalInput").ap()
    w = nc.dram_tensor("w", [IN, F], BF16, kind="ExternalInput").ap()
    dvt = nc.dram_tensor("dvt", [P, E // P], F32, kind="ExternalInput").ap()
    det = nc.dram_tensor("det", [P, NJ], F32, kind="ExternalInput").ap()
    out = nc.dram_tensor("out", [NS, F], F32, kind="ExternalOutput").ap()

    with tile.TileContext(nc) as tc, ExitStack() as ctx:
        consts = ctx.enter_context(tc.tile_pool(name="consts", bufs=1))
        sbig = ctx.enter_context(tc.tile_pool(name="sbig", bufs=1))
        mtp = ctx.enter_context(tc.tile_pool(name="mtp", bufs=4))
        mtT_p = ctx.enter_context(tc.tile_pool(name="mtT", bufs=4))
        eyp_p = ctx.enter_context(tc.tile_pool(name="eyp", bufs=3))
        eyf_p = ctx.enter_context(tc.tile_pool(name="eyf", bufs=2))
        z_p = ctx.enter_context(tc.tile_pool(name="zp", bufs=3))
        ps_tr = ctx.enter_context(tc.tile_pool(name="ps_tr", bufs=4, space="PSUM"))
        ps_ey = ctx.enter_context(tc.tile_pool(name="ps_ey", bufs=1, space="PSUM"))
        ps_p2 = ctx.enter_context(tc.tile_pool(name="ps_p2", bufs=2, space="PSUM"))
        dram = ctx.enter_context(tc.tile_pool(name="dram", bufs=4, space="DRAM"))

        id16 = consts.tile([P, P], BF16, tag="id16")
        id32 = consts.tile([P, P], F32, tag="id32")
        make_identity(nc, id16[:])
        make_identity(nc, id32[:])

        # Prefetch the first MT superblock ahead of everything else so
        # phase 1 can start as soon as possible.
        mt0_sb = mtp.tile([P, ET, NS], BF16, tag="mt")
        nc.sync.dma_start(
            mt0_sb[:], mt[0:EB, :].rearrange("(t p) n -> p t n", p=P))

        # Tiny warm-up AllReduce: the CC channel pays ~50us of one-time
        # init on its first collective; issue it at t=0 so the init runs
        # under step A / p1(0) instead of delaying the first real AR.
        warm = consts.tile([P, 8], BF16, tag="warm")
        nc.vector.memset(warm[:], 0.0)
        win_t = dram.tile([P, 8], BF16, tag="win")
        wout_t = dram.tile([P, 8], BF16, tag="wout")
        nc.scalar.dma_start(win_t[:], warm[:])
        nc.gpsimd.collective_compute(
            "AllReduce",
            mybir.AluOpType.add,
            replica_groups=[list(range(CORES))],
            ins=[win_t.opt()],
            outs=[wout_t.opt()],
        )

        # xsT loaded pre-transposed straight off the XBAR: [in_ft, nodes]
        xsT_sb = sbig.tile([P, KI, NS], BF16, tag="xsT")
        for k in range(KI):
            nc.sync.dma_start(
                xsT_sb[:, k, :], xs[:, k * P:(k + 1) * P], transpose=True)

        w_sb = consts.tile([P, KI, F], BF16, tag="w")
        nc.sync.dma_start(w_sb[:], w.rearrange("(k p) f -> p k f", p=P))
        dvt_sb = consts.tile([P, E // P], F32, tag="dvt")
        nc.sync.dma_start(dvt_sb[:], dvt)
        det_sb = consts.tile([P, NJ], F32, tag="det")
        nc.sync.dma_start(det_sb[:], det)

        y_sb = sbig.tile([P, NS], BF16, tag="y")
        ny_sb = sbig.tile([P, NS], F32, tag="ny_sb")

        # Copy-engine alternation between DVE and ACT to split PSUM->SBUF load.
        cp_state = [0]

        def copy_eng():
            cp_state[0] ^= 1
            if cp_state[0]:
                return nc.vector.tensor_copy
            return nc.scalar.copy

        # ---- Step A: y = xs @ w ------------------------------------------
        for i in range(NJ):
            yp = ps_tr.tile([P, F], F32, tag="tr")
            for k in range(KI):
                nc.tensor.matmul(
                    yp[:],
                    lhsT=xsT_sb[:, k, i * P:(i + 1) * P],
                    rhs=w_sb[:, k, :],
                    start=(k == 0),
                    stop=(k == KI - 1),
                )
            copy_eng()(y_sb[:, i * P:(i + 1) * P], yp[:])

        # ---- Phase 1 per superblock --------------------------------------
        def emit_p1_block(g, mt_sb=None):
            if mt_sb is None:
                mt_sb = mtp.tile([P, ET, NS], BF16, tag="mt")
                nc.sync.dma_start(
                    mt_sb[:],
                    mt[g * EB:(g + 1) * EB, :].rearrange(
                        "(t p) n -> p t n", p=P),
                )
            eyT = ps_ey.tile([P, EB], F32, tag="ey")

            def p1_transpose(j):
                mtT = mtT_p.tile([P, EB], BF16, tag="mtT")
                for h in range(EH):
                    nch = min(4, ET - h * 4)
                    tr = ps_tr.tile([P, 512], BF16, tag="tr")
                    for c in range(nch):
                        t = h * 4 + c
                        nc.tensor.transpose(
                            tr[:, c * P:(c + 1) * P],
                            mt_sb[:, t, j * P:(j + 1) * P],
                            id16[:],
                        )
                    copy_eng()(
                        mtT[:, h * EW:h * EW + nch * P], tr[:, : nch * P])
                return mtT

            def p1_matmul(j, mtT):
                for h in range(EH):
                    hw = min(EW, EB - h * EW)
                    nc.tensor.matmul(
                        eyT[:, h * EW:h * EW + hw],
                        lhsT=y_sb[:, j * P:(j + 1) * P],
                        rhs=mtT[:, h * EW:h * EW + hw],
                        start=(j == 0),
                        stop=(j == NJ - 1),
                    )

            mtT_prev = None
            for j in range(NJ):
                mtT_cur = p1_transpose(j)
                if mtT_prev is not None:
                    p1_matmul(j - 1, mtT_prev)
                mtT_prev = mtT_cur
            p1_matmul(NJ - 1, mtT_prev)

            eyp = eyp_p.tile([P, EB], BF16, tag="eyp")
            for h in range(EH):
                hw = min(EW, EB - h * EW)
                nc.vector.tensor_copy(
                    eyp[:, h * EW:h * EW + hw], eyT[:, h * EW:h * EW + hw])
            return mt_sb, eyp

        # ---- Phase 2 per superblock --------------------------------------
        def emit_p2_block(g, mt_sb, eyf, off):
            z = z_p.tile([P, EB], BF16, tag="z")
            for h in range(EH):
                nch = min(4, ET - h * 4)
                tr = ps_tr.tile([P, 512], BF16, tag="tr")
                for c in range(nch):
                    t = h * 4 + c
                    nc.tensor.transpose(
                        tr[:, c * P:(c + 1) * P],
                        eyf[:, off + t * P:off + (t + 1) * P],
                        id16[:],
                    )
                for c in range(nch):
                    t = h * 4 + c
                    nc.vector.tensor_scalar_mul(
                        z[:, t * P:(t + 1) * P],
                        tr[:, c * P:(c + 1) * P],
                        dvt_sb[:, g * ET + t:g * ET + t + 1],
                    )
            for q in range(NQ):
                qw = min(NW, NS - q * NW)
                p2 = ps_p2.tile([P, NW], F32, tag="p2")
                for t in range(ET):
                    nc.tensor.matmul(
                        p2[:, :qw],
                        lhsT=z[:, t * P:(t + 1) * P],
                        rhs=mt_sb[:, t, q * NW:q * NW + qw],
                        start=(t == 0),
                        stop=(t == ET - 1),
                    )
                if g == 0:
                    nc.vector.tensor_copy(
                        ny_sb[:, q * NW:q * NW + qw], p2[:, :qw])
                else:
                    nc.vector.tensor_add(
                        ny_sb[:, q * NW:q * NW + qw],
                        ny_sb[:, q * NW:q * NW + qw],
                        p2[:, :qw],
                    )

        # ---- AllReduce of one superblock group ---------------------------
        def emit_ar_group(eyps):
            s = len(eyps)
            bin_t = dram.tile([P, s * EB], BF16, tag=f"bin{s}")
            bout_t = dram.tile([P, s * EB], BF16, tag=f"bout{s}")
            for i, eyp in enumerate(eyps):
                nc.sync.dma_start(bin_t[:, i * EB:(i + 1) * EB], eyp[:])
            nc.gpsimd.collective_compute(
                "AllReduce",
                mybir.AluOpType.add,
                replica_groups=[list(range(CORES))],
                ins=[bin_t.opt()],
                outs=[bout_t.opt()],
            )
            # Result load waits on the collective — issue it from the ACT
            # HWDGE so its stalled descriptors can't block the SP-issued
            # mt-load stream.
            eyf = eyf_p.tile([P, s * EB], BF16, tag=f"eyf{s}")
            nc.scalar.dma_start(eyf[:], bout_t[:])
            return eyf

        mts = {}
        eyfs = {}

        def emit_p2_group(gi):
            group = AR_GROUPS[gi]
            for i, g in enumerate(group):
                emit_p2_block(g, mts[g], eyfs[gi], i * EB)

        for gi, group in enumerate(AR_GROUPS):
            eyps = []
            for g in group:
                mt_sb, eyp = emit_p1_block(g, mt0_sb if g == 0 else None)
                mts[g] = mt_sb
                eyps.append(eyp)
            eyfs[gi] = emit_ar_group(eyps)
            if gi >= 1:
                emit_p2_group(gi - 1)
        emit_p2_group(len(AR_GROUPS) - 1)

        # ---- Finalize: out = det ⊙ ny_sb.T -------------------------------
        out_sb = sbig.tile([P, NS], F32, tag="out_sb")
        for i0 in range(0, NJ, 4):
            nch = min(4, NJ - i0)
            tr = ps_tr.tile([P, 512], F32, tag="tr")
            for c in range(nch):
                i = i0 + c
                nc.tensor.transpose(
                    tr[:, c * P:(c + 1) * P],
                    ny_sb[:, i * P:(i + 1) * P],
                    id32[:],
                )
            for c in range(nch):
                i = i0 + c
                nc.vector.tensor_scalar_mul(
                    out_sb[:, i * P:(i + 1) * P],
                    tr[:, c * P:(c + 1) * P],
                    det_sb[:, i:i + 1],
                )
        nc.sync.dma_start(
            out.rearrange("(i p) f -> p i f", p=P),
            out_sb[:].rearrange("p (i f) -> p i f", f=F))

    return nc


@functools.lru_cache(maxsize=2)
def _compiled(cfg_items):
    cfg = dict(cfg_items)
    nc = bacc.Bacc(
        "TRN2",
        target_bir_lowering=False,
        debug=False,
        num_devices=cfg["CORES"],
    )
    build_kernel(nc, cfg)
    nc.compile()
    return nc


def shard_inputs(x, weight, MT, D_v_diag, D_e_diag, cfg):
    """Host-side sharding + dtype prep. Returns in_maps for the 8 cores."""
    N, E, IN, F, CORES = cfg["N"], cfg["E"], cfg["IN"], cfg["F"], cfg["CORES"]
    NS = N // CORES
    bf = ml_dtypes.bfloat16
    w_b = np.ascontiguousarray(np.asarray(weight, dtype=np.float32)).astype(bf)
    x_f = np.asarray(x, dtype=np.float32)
    mt_f = np.asarray(MT, dtype=np.float32)
    dv = np.asarray(D_v_diag, dtype=np.float32)
    de = np.asarray(D_e_diag, dtype=np.float32)
    # [P, E/P] with chunk index on the free axis
    dvt = np.ascontiguousarray(dv.reshape(E // 128, 128).T)
    in_maps = []
    for c in range(CORES):
        sl = slice(c * NS, (c + 1) * NS)
        det = np.ascontiguousarray(
            (0.5 * de[sl]).reshape(NS // 128, 128).T)
        in_maps.append({
            "mt": np.ascontiguousarray(mt_f[:, sl]).astype(bf),
            "xs": np.ascontiguousarray(x_f[sl]).astype(bf),
            "w": w_b,
            "dvt": dvt,
            "det": det,
        })
    return in_maps


def _run(x, weight, MT, D_v_diag, D_e_diag, cfg=None, trace=False):
    cfg = cfg or FULL_CFG
    nc = _compiled(tuple(sorted(cfg.items())))
    in_maps = shard_inputs(x, weight, MT, D_v_diag, D_e_diag, cfg)
    res = run_bass_kernel_spmd(
        nc, in_maps, core_ids=list(range(cfg["CORES"])), trace=trace)
    NS = cfg["N"] // cfg["CORES"]
    out = np.concatenate(
        [np.asarray(res.results[c]["out"]) for c in range(cfg["CORES"])],
        axis=0,
    ).astype(np.float32)
    return out, res


def kernel(x, weight, MT, D_v_diag, D_e_diag):
    out, _ = _run(x, weight, MT, D_v_diag, D_e_diag)
    return out


# revision 9
# speedup vs baseline: 1.1214x; 1.1214x over previous
"""HGNN conv distributed Bass kernel for 8 TRN2 NeuronCores.

Computes  out = 0.5 * D_e ⊙ (MT.T @ (D_v ⊙ (MT @ (x @ W))))
with N=16384 nodes, E=8192 hyperedges, IN_FT=256, OUT_FT=128.

Sharding (node/data parallel per hint): MT columns, x rows and D_e are
sharded over nodes across the 8 cores; W and D_v are replicated. The
MT @ y contraction over nodes becomes a partial sum + AllReduce; the
MT.T @ z contraction over edges is local per node shard.

Per core the MT shard [E, N/8] streams through SBUF once (bf16,
host-cast) in 8 superblocks of EB=1024 edges. Phase 1 produces eyT
partials via PE-transposes of MT tiles; partials are AllReduced in 5
uneven groups of superblocks [1,2,2,2,1] so the first collective
starts early and the last leaves only one superblock of phase-2 work
after it — the serialized collective channel (~130us) hides under the
~220us of PE work. Phase 2 re-uses the natural MT tiles from SBUF with
the reduced z as stationary, accumulating nyT across superblocks.
x is loaded pre-transposed via the DMA XBAR so step A needs no PE
transposes.
"""

import functools
from contextlib import ExitStack

import ml_dtypes
import numpy as np

import concourse.bass as bass
import concourse.mybir as mybir
import concourse.tile as tile
from concourse import bacc
from concourse.bass_utils import run_bass_kernel_spmd
from concourse.masks import make_identity

P = 128
BF16 = mybir.dt.bfloat16
F32 = mybir.dt.float32

FULL_CFG = dict(N=16384, E=8192, IN=256, F=128, CORES=8, G=8)

# Superblocks per AllReduce call; uneven so the collective chain starts
# early and drains before the phase-2 tail.
AR_GROUPS = ((0, 1), (2, 3), (4, 5), (6, 7))


def _ceil_div(a, b):
    return -(-a // b)


def build_kernel(nc, cfg):
    N, E, IN, F, CORES, G = (
        cfg["N"], cfg["E"], cfg["IN"], cfg["F"], cfg["CORES"], cfg["G"])
    NS = N // CORES          # nodes per core
    EB = E // G              # edges per superblock
    ET = EB // P             # 128-edge chunks per superblock
    NJ = NS // P             # 128-node chunks (phase-1 contraction)
    KI = IN // P             # 128-in_ft chunks
    EH = _ceil_div(EB, 512)  # 512-edge groups per superblock (phase-1 psum)
    NQ = _ceil_div(NS, 512)  # 512-node groups (phase-2 free dim)
    EW = min(EB, 512)        # phase-1 psum group width
    NW = min(NS, 512)        # phase-2 moving free width
    assert EB % P == 0 and NS % P == 0 and IN % P == 0 and F == P

    mt = nc.dram_tensor("mt", [E, NS], BF16, kind="ExternalInput").ap()
    xs = nc.dram_tensor("xs", [NS, IN], BF16, kind="ExternalInput").ap()
    w = nc.dram_tensor("w", [IN, F], BF16, kind="ExternalInput").ap()
    dvt = nc.dram_tensor("dvt", [P, E // P], F32, kind="ExternalInput").ap()
    det = nc.dram_tensor("det", [P, NJ], F32, kind="ExternalInput").ap()
    out = nc.dram_tensor("out", [NS, F], F32, kind="ExternalOutput").ap()

    with tile.TileContext(nc) as tc, ExitStack() as ctx:
        consts = ctx.enter_context(tc.tile_pool(name="consts", bufs=1))
        sbig = ctx.enter_context(tc.tile_pool(name="sbig", bufs=1))
        mtp = ctx.enter_context(tc.tile_pool(name="mtp", bufs=4))
        mtT_p = ctx.enter_context(tc.tile_pool(name="mtT", bufs=4))
        eyp_p = ctx.enter_context(tc.tile_pool(name="eyp", bufs=3))
        eyf_p = ctx.enter_context(tc.tile_pool(name="eyf", bufs=2))
        z_p = ctx.enter_context(tc.tile_pool(name="zp", bufs=3))
        ps_tr = ctx.enter_context(tc.tile_pool(name="ps_tr", bufs=4, space="PSUM"))
        ps_ey = ctx.enter_context(tc.tile_pool(name="ps_ey", bufs=1, space="PSUM"))
        ps_p2 = ctx.enter_context(tc.tile_pool(name="ps_p2", bufs=2, space="PSUM"))
        dram = ctx.enter_context(tc.tile_pool(name="dram", bufs=4, space="DRAM"))

        id16 = consts.tile([P, P], BF16, tag="id16")
        id32 = consts.tile([P, P], F32, tag="id32")
        make_identity(nc, id16[:])
        make_identity(nc, id32[:])

        # Prefetch the first MT superblock ahead of everything else so
        # phase 1 can start as soon as possible.
        mt0_sb = mtp.tile([P, ET, NS], BF16, tag="mt")
        nc.sync.dma_start(
            mt0_sb[:], mt[0:EB, :].rearrange("(t p) n -> p t n", p=P))

        # xsT loaded pre-transposed straight off the XBAR: [in_ft, nodes]
        xsT_sb = sbig.tile([P, KI, NS], BF16, tag="xsT")
        for k in range(KI):
            nc.sync.dma_start(
                xsT_sb[:, k, :], xs[:, k * P:(k + 1) * P], transpose=True)

        w_sb = consts.tile([P, KI, F], BF16, tag="w")
        nc.sync.dma_start(w_sb[:], w.rearrange("(k p) f -> p k f", p=P))
        dvt_sb = consts.tile([P, E // P], F32, tag="dvt")
        nc.sync.dma_start(dvt_sb[:], dvt)
        det_sb = consts.tile([P, NJ], F32, tag="det")
        nc.sync.dma_start(det_sb[:], det)

        y_sb = sbig.tile([P, NS], BF16, tag="y")
        ny_sb = sbig.tile([P, NS], F32, tag="ny_sb")

        # Copy-engine alternation between DVE and ACT to split PSUM->SBUF load.
        cp_state = [0]

        def copy_eng():
            cp_state[0] ^= 1
            if cp_state[0]:
                return nc.vector.tensor_copy
            return nc.scalar.copy

        # ---- Step A: y = xs @ w ------------------------------------------
        for i in range(NJ):
            yp = ps_tr.tile([P, F], F32, tag="tr")
            for k in range(KI):
                nc.tensor.matmul(
                    yp[:],
                    lhsT=xsT_sb[:, k, i * P:(i + 1) * P],
                    rhs=w_sb[:, k, :],
                    start=(k == 0),
                    stop=(k == KI - 1),
                )
            copy_eng()(y_sb[:, i * P:(i + 1) * P], yp[:])

        # ---- Phase 1 per superblock --------------------------------------
        def emit_p1_block(g, mt_sb=None):
            if mt_sb is None:
                mt_sb = mtp.tile([P, ET, NS], BF16, tag="mt")
                nc.sync.dma_start(
                    mt_sb[:],
                    mt[g * EB:(g + 1) * EB, :].rearrange(
                        "(t p) n -> p t n", p=P),
                )
            eyT = ps_ey.tile([P, EB], F32, tag="ey")

            def p1_transpose(j):
                mtT = mtT_p.tile([P, EB], BF16, tag="mtT")
                for h in range(EH):
                    nch = min(4, ET - h * 4)
                    tr = ps_tr.tile([P, 512], BF16, tag="tr")
                    for c in range(nch):
                        t = h * 4 + c
                        nc.tensor.transpose(
                            tr[:, c * P:(c + 1) * P],
                            mt_sb[:, t, j * P:(j + 1) * P],
                            id16[:],
                        )
                    copy_eng()(
                        mtT[:, h * EW:h * EW + nch * P], tr[:, : nch * P])
                return mtT

            def p1_matmul(j, mtT):
                for h in range(EH):
                    hw = min(EW, EB - h * EW)
                    nc.tensor.matmul(
                        eyT[:, h * EW:h * EW + hw],
                        lhsT=y_sb[:, j * P:(j + 1) * P],
                        rhs=mtT[:, h * EW:h * EW + hw],
                        start=(j == 0),
                        stop=(j == NJ - 1),
                    )

            mtT_prev = None
            for j in range(NJ):
                mtT_cur = p1_transpose(j)
                if mtT_prev is not None:
                    p1_matmul(j - 1, mtT_prev)
                mtT_prev = mtT_cur
            p1_matmul(NJ - 1, mtT_prev)

            eyp = eyp_p.tile([P, EB], BF16, tag="eyp")
            for h in range(EH):
                hw = min(EW, EB - h * EW)
                nc.vector.tensor_copy(
                    eyp[:, h * EW:h * EW + hw], eyT[:, h * EW:h * EW + hw])
            return mt_sb, eyp

        # ---- Phase 2 per superblock --------------------------------------
        def emit_p2_block(g, mt_sb, eyf, off):
            z = z_p.tile([P, EB], BF16, tag="z")
            for h in range(EH):
                nch = min(4, ET - h * 4)
                tr = ps_tr.tile([P, 512], BF16, tag="tr")
                for c in range(nch):
                    t = h * 4 + c
                    nc.tensor.transpose(
                        tr[:, c * P:(c + 1) * P],
                        eyf[:, off + t * P:off + (t + 1) * P],
                        id16[:],
                    )
                for c in range(nch):
                    t = h * 4 + c
                    nc.vector.tensor_scalar_mul(
                        z[:, t * P:(t + 1) * P],
                        tr[:, c * P:(c + 1) * P],
                        dvt_sb[:, g * ET + t:g * ET + t + 1],
                    )
            for q in range(NQ):
                qw = min(NW, NS - q * NW)
                p2 = ps_p2.tile([P, NW], F32, tag="p2")
                for t in range(ET):
                    nc.tensor.matmul(
                        p2[:, :qw],
                        lhsT=z[:, t * P:(t + 1) * P],
                        rhs=mt_sb[:, t, q * NW:q * NW + qw],
                        start=(t == 0),
                        stop=(t == ET - 1),
                    )
                if g == 0:
                    nc.vector.tensor_copy(
                        ny_sb[:, q * NW:q * NW + qw], p2[:, :qw])
                else:
                    nc.vector.tensor_add(
                        ny_sb[:, q * NW:q * NW + qw],
                        ny_sb[:, q * NW:q * NW + qw],
                        p2[:, :qw],
                    )

        # ---- AllReduce of one superblock group ---------------------------
        def emit_ar_group(eyps):
            s = len(eyps)
            bin_t = dram.tile([P, s * EB], BF16, tag=f"bin{s}")
            bout_t = dram.tile([P, s * EB], BF16, tag=f"bout{s}")
            # bin stores on the ACT HWDGE: they gate AR dispatch and must
            # not queue behind the big SP-issued mt prefetches.
            for i, eyp in enumerate(eyps):
                nc.scalar.dma_start(bin_t[:, i * EB:(i + 1) * EB], eyp[:])
            nc.gpsimd.collective_compute(
                "AllReduce",
                mybir.AluOpType.add,
                replica_groups=[list(range(CORES))],
                ins=[bin_t.opt()],
                outs=[bout_t.opt()],
            )
            # Result load waits on the collective — issue it from the ACT
            # HWDGE so its stalled descriptors can't block the SP-issued
            # mt-load stream.
            eyf = eyf_p.tile([P, s * EB], BF16, tag=f"eyf{s}")
            nc.scalar.dma_start(eyf[:], bout_t[:])
            return eyf

        mts = {}
        eyfs = {}

        def emit_p2_group(gi):
            group = AR_GROUPS[gi]
            for i, g in enumerate(group):
                emit_p2_block(g, mts[g], eyfs[gi], i * EB)

        for gi, group in enumerate(AR_GROUPS):
            eyps = []
            for g in group:
                mt_sb, eyp = emit_p1_block(g, mt0_sb if g == 0 else None)
                mts[g] = mt_sb
                eyps.append(eyp)
            eyfs[gi] = emit_ar_group(eyps)
            if gi >= 1:
                emit_p2_group(gi - 1)
        emit_p2_group(len(AR_GROUPS) - 1)

        # ---- Finalize: out = det ⊙ ny_sb.T -------------------------------
        out_sb = sbig.tile([P, NS], F32, tag="out_sb")
        for i0 in range(0, NJ, 4):
            nch = min(4, NJ - i0)
            tr = ps_tr.tile([P, 512], F32, tag="tr")
            for c in range(nch):
                i = i0 + c
                nc.tensor.transpose(
                    tr[:, c * P:(c + 1) * P],
                    ny_sb[:, i * P:(i + 1) * P],
                    id32[:],
                )
            for c in range(nch):
                i = i0 + c
                nc.vector.tensor_scalar_mul(
                    out_sb[:, i * P:(i + 1) * P],
                    tr[:, c * P:(c + 1) * P],
                    det_sb[:, i:i + 1],
                )
        nc.sync.dma_start(
            out.rearrange("(i p) f -> p i f", p=P),
            out_sb[:].rearrange("p (i f) -> p i f", f=F))

    return nc


@functools.lru_cache(maxsize=2)
def _compiled(cfg_items):
    cfg = dict(cfg_items)
    nc = bacc.Bacc(
        "TRN2",
        target_bir_lowering=False,
        debug=False,
        num_devices=cfg["CORES"],
    )
    build_kernel(nc, cfg)
    nc.compile()
    return nc


def shard_inputs(x, weight, MT, D_v_diag, D_e_diag, cfg):
    """Host-side sharding + dtype prep. Returns in_maps for the 8 cores."""
    N, E, IN, F, CORES = cfg["N"], cfg["E"], cfg["IN"], cfg["F"], cfg["CORES"]
    NS = N // CORES
    bf = ml_dtypes.bfloat16
    w_b = np.ascontiguousarray(np.asarray(weight, dtype=np.float32)).astype(bf)
    x_f = np.asarray(x, dtype=np.float32)
    mt_f = np.asarray(MT, dtype=np.float32)
    dv = np.asarray(D_v_diag, dtype=np.float32)
    de = np.asarray(D_e_diag, dtype=np.float32)
    # [P, E/P] with chunk index on the free axis
    dvt = np.ascontiguousarray(dv.reshape(E // 128, 128).T)
    in_maps = []
    for c in range(CORES):
        sl = slice(c * NS, (c + 1) * NS)
        det = np.ascontiguousarray(
            (0.5 * de[sl]).reshape(NS // 128, 128).T)
        in_maps.append({
            "mt": np.ascontiguousarray(mt_f[:, sl]).astype(bf),
            "xs": np.ascontiguousarray(x_f[sl]).astype(bf),
            "w": w_b,
            "dvt": dvt,
            "det": det,
        })
    return in_maps


def _run(x, weight, MT, D_v_diag, D_e_diag, cfg=None, trace=False):
    cfg = cfg or FULL_CFG
    nc = _compiled(tuple(sorted(cfg.items())))
    in_maps = shard_inputs(x, weight, MT, D_v_diag, D_e_diag, cfg)
    res = run_bass_kernel_spmd(
        nc, in_maps, core_ids=list(range(cfg["CORES"])), trace=trace)
    NS = cfg["N"] // cfg["CORES"]
    out = np.concatenate(
        [np.asarray(res.results[c]["out"]) for c in range(cfg["CORES"])],
        axis=0,
    ).astype(np.float32)
    return out, res


def kernel(x, weight, MT, D_v_diag, D_e_diag):
    out, _ = _run(x, weight, MT, D_v_diag, D_e_diag)
    return out


# revision 12
# speedup vs baseline: 1.1410x; 1.0175x over previous
"""HGNN conv distributed Bass kernel for 8 TRN2 NeuronCores.

Computes  out = 0.5 * D_e ⊙ (MT.T @ (D_v ⊙ (MT @ (x @ W))))
with N=16384 nodes, E=8192 hyperedges, IN_FT=256, OUT_FT=128.

Sharding (node/data parallel per hint): MT columns, x rows and D_e are
sharded over nodes across the 8 cores; W and D_v are replicated. The
MT @ y contraction over nodes becomes a partial sum + AllReduce; the
MT.T @ z contraction over edges is local per node shard.

Per core the MT shard [E, N/8] streams through SBUF once (bf16,
host-cast) in 8 superblocks of EB=1024 edges. Phase 1 produces eyT
partials via PE-transposes of MT tiles; partials are AllReduced in 5
uneven groups of superblocks [1,2,2,2,1] so the first collective
starts early and the last leaves only one superblock of phase-2 work
after it — the serialized collective channel (~130us) hides under the
~220us of PE work. Phase 2 re-uses the natural MT tiles from SBUF with
the reduced z as stationary, accumulating nyT across superblocks.
x is loaded pre-transposed via the DMA XBAR so step A needs no PE
transposes.
"""

import functools
from contextlib import ExitStack

import ml_dtypes
import numpy as np

import concourse.bass as bass
import concourse.mybir as mybir
import concourse.tile as tile
from concourse import bacc
from concourse.bass_utils import run_bass_kernel_spmd
from concourse.masks import make_identity

P = 128
BF16 = mybir.dt.bfloat16
F32 = mybir.dt.float32

FULL_CFG = dict(N=16384, E=8192, IN=256, F=128, CORES=8, G=8)

# Superblocks per AllReduce call; uneven so the collective chain starts
# early and drains before the phase-2 tail.
AR_GROUPS = ((0, 1), (2, 3), (4, 5), (6, 7))


def _ceil_div(a, b):
    return -(-a // b)


def build_kernel(nc, cfg):
    N, E, IN, F, CORES, G = (
        cfg["N"], cfg["E"], cfg["IN"], cfg["F"], cfg["CORES"], cfg["G"])
    NS = N // CORES          # nodes per core
    EB = E // G              # edges per superblock
    ET = EB // P             # 128-edge chunks per superblock
    NJ = NS // P             # 128-node chunks (phase-1 contraction)
    KI = IN // P             # 128-in_ft chunks
    EH = _ceil_div(EB, 512)  # 512-edge groups per superblock (phase-1 psum)
    NQ = _ceil_div(NS, 512)  # 512-node groups (phase-2 free dim)
    EW = min(EB, 512)        # phase-1 psum group width
    NW = min(NS, 512)        # phase-2 moving free width
    assert EB % P == 0 and NS % P == 0 and IN % P == 0 and F == P

    mt = nc.dram_tensor("mt", [E, NS], BF16, kind="ExternalInput").ap()
    xs = nc.dram_tensor("xs", [NS, IN], BF16, kind="ExternalInput").ap()
    w = nc.dram_tensor("w", [IN, F], BF16, kind="ExternalInput").ap()
    dvt = nc.dram_tensor("dvt", [P, E // P], F32, kind="ExternalInput").ap()
    det = nc.dram_tensor("det", [P, NJ], F32, kind="ExternalInput").ap()
    out = nc.dram_tensor("out", [NS, F], F32, kind="ExternalOutput").ap()

    with tile.TileContext(nc) as tc, ExitStack() as ctx:
        consts = ctx.enter_context(tc.tile_pool(name="consts", bufs=1))
        sbig = ctx.enter_context(tc.tile_pool(name="sbig", bufs=1))
        mtp = ctx.enter_context(tc.tile_pool(name="mtp", bufs=4))
        mtT_p = ctx.enter_context(tc.tile_pool(name="mtT", bufs=4))
        eyp_p = ctx.enter_context(tc.tile_pool(name="eyp", bufs=3))
        eyf_p = ctx.enter_context(tc.tile_pool(name="eyf", bufs=4))
        z_p = ctx.enter_context(tc.tile_pool(name="zp", bufs=3))
        ps_tr = ctx.enter_context(tc.tile_pool(name="ps_tr", bufs=4, space="PSUM"))
        ps_ey = ctx.enter_context(tc.tile_pool(name="ps_ey", bufs=1, space="PSUM"))
        ps_p2 = ctx.enter_context(tc.tile_pool(name="ps_p2", bufs=2, space="PSUM"))
        dram = ctx.enter_context(tc.tile_pool(name="dram", bufs=4, space="DRAM"))

        id16 = consts.tile([P, P], BF16, tag="id16")
        id32 = consts.tile([P, P], F32, tag="id32")
        make_identity(nc, id16[:])
        make_identity(nc, id32[:])

        # Prefetch the first MT superblock ahead of everything else so
        # phase 1 can start as soon as possible.
        mt0_sb = mtp.tile([P, ET, NS], BF16, tag="mt")
        nc.sync.dma_start(
            mt0_sb[:], mt[0:EB, :].rearrange("(t p) n -> p t n", p=P))

        # xsT loaded pre-transposed straight off the XBAR: [in_ft, nodes]
        xsT_sb = sbig.tile([P, KI, NS], BF16, tag="xsT")
        for k in range(KI):
            nc.sync.dma_start(
                xsT_sb[:, k, :], xs[:, k * P:(k + 1) * P], transpose=True)

        w_sb = consts.tile([P, KI, F], BF16, tag="w")
        nc.sync.dma_start(w_sb[:], w.rearrange("(k p) f -> p k f", p=P))
        dvt_sb = consts.tile([P, E // P], F32, tag="dvt")
        nc.sync.dma_start(dvt_sb[:], dvt)
        det_sb = consts.tile([P, NJ], F32, tag="det")
        nc.sync.dma_start(det_sb[:], det)

        y_sb = sbig.tile([P, NS], BF16, tag="y")
        ny_sb = sbig.tile([P, NS], F32, tag="ny_sb")

        # Copy-engine alternation between DVE and ACT to split PSUM->SBUF load.
        cp_state = [0]

        def copy_eng():
            cp_state[0] ^= 1
            if cp_state[0]:
                return nc.vector.tensor_copy
            return nc.scalar.copy

        # ---- Step A: y = xs @ w ------------------------------------------
        for i in range(NJ):
            yp = ps_tr.tile([P, F], F32, tag="tr")
            for k in range(KI):
                nc.tensor.matmul(
                    yp[:],
                    lhsT=xsT_sb[:, k, i * P:(i + 1) * P],
                    rhs=w_sb[:, k, :],
                    start=(k == 0),
                    stop=(k == KI - 1),
                )
            copy_eng()(y_sb[:, i * P:(i + 1) * P], yp[:])

        # ---- Phase 1 per superblock --------------------------------------
        def emit_p1_block(g, mt_sb=None):
            if mt_sb is None:
                mt_sb = mtp.tile([P, ET, NS], BF16, tag="mt")
                nc.sync.dma_start(
                    mt_sb[:],
                    mt[g * EB:(g + 1) * EB, :].rearrange(
                        "(t p) n -> p t n", p=P),
                )
            eyT = ps_ey.tile([P, EB], F32, tag="ey")

            def p1_transpose(j):
                mtT = mtT_p.tile([P, EB], BF16, tag="mtT")
                for h in range(EH):
                    nch = min(4, ET - h * 4)
                    tr = ps_tr.tile([P, 512], BF16, tag="tr")
                    for c in range(nch):
                        t = h * 4 + c
                        nc.tensor.transpose(
                            tr[:, c * P:(c + 1) * P],
                            mt_sb[:, t, j * P:(j + 1) * P],
                            id16[:],
                        )
                    copy_eng()(
                        mtT[:, h * EW:h * EW + nch * P], tr[:, : nch * P])
                return mtT

            def p1_matmul(j, mtT):
                for h in range(EH):
                    hw = min(EW, EB - h * EW)
                    nc.tensor.matmul(
                        eyT[:, h * EW:h * EW + hw],
                        lhsT=y_sb[:, j * P:(j + 1) * P],
                        rhs=mtT[:, h * EW:h * EW + hw],
                        start=(j == 0),
                        stop=(j == NJ - 1),
                    )

            mtT_prev = None
            for j in range(NJ):
                mtT_cur = p1_transpose(j)
                if mtT_prev is not None:
                    p1_matmul(j - 1, mtT_prev)
                mtT_prev = mtT_cur
            p1_matmul(NJ - 1, mtT_prev)

            eyp = eyp_p.tile([P, EB], BF16, tag="eyp")
            for h in range(EH):
                hw = min(EW, EB - h * EW)
                nc.vector.tensor_copy(
                    eyp[:, h * EW:h * EW + hw], eyT[:, h * EW:h * EW + hw])
            return mt_sb, eyp

        # ---- Phase 2 per superblock --------------------------------------
        # p2 re-streams its MT superblock from HBM: this decouples p2 from
        # p1's SBUF liveness, letting all of p1 run back-to-back so the
        # AllReduce chain is never input-starved.
        def emit_p2_block(g, eyf, off):
            mt_sb = mtp.tile([P, ET, NS], BF16, tag="mt")
            nc.sync.dma_start(
                mt_sb[:],
                mt[g * EB:(g + 1) * EB, :].rearrange("(t p) n -> p t n", p=P),
            )
            z = z_p.tile([P, EB], BF16, tag="z")
            for h in range(EH):
                nch = min(4, ET - h * 4)
                tr = ps_tr.tile([P, 512], BF16, tag="tr")
                for c in range(nch):
                    t = h * 4 + c
                    nc.tensor.transpose(
                        tr[:, c * P:(c + 1) * P],
                        eyf[:, off + t * P:off + (t + 1) * P],
                        id16[:],
                    )
                for c in range(nch):
                    t = h * 4 + c
                    nc.vector.tensor_scalar_mul(
                        z[:, t * P:(t + 1) * P],
                        tr[:, c * P:(c + 1) * P],
                        dvt_sb[:, g * ET + t:g * ET + t + 1],
                    )
            for q in range(NQ):
                qw = min(NW, NS - q * NW)
                p2 = ps_p2.tile([P, NW], F32, tag="p2")
                for t in range(ET):
                    nc.tensor.matmul(
                        p2[:, :qw],
                        lhsT=z[:, t * P:(t + 1) * P],
                        rhs=mt_sb[:, t, q * NW:q * NW + qw],
                        start=(t == 0),
                        stop=(t == ET - 1),
                    )
                if g == 0:
                    nc.vector.tensor_copy(
                        ny_sb[:, q * NW:q * NW + qw], p2[:, :qw])
                else:
                    nc.vector.tensor_add(
                        ny_sb[:, q * NW:q * NW + qw],
                        ny_sb[:, q * NW:q * NW + qw],
                        p2[:, :qw],
                    )

        # ---- AllReduce of one superblock group ---------------------------
        def emit_ar_group(eyps):
            s = len(eyps)
            bin_t = dram.tile([P, s * EB], BF16, tag=f"bin{s}")
            bout_t = dram.tile([P, s * EB], BF16, tag=f"bout{s}")
            # bin stores on the ACT HWDGE: they gate AR dispatch and must
            # not queue behind the big SP-issued mt prefetches.
            for i, eyp in enumerate(eyps):
                nc.scalar.dma_start(bin_t[:, i * EB:(i + 1) * EB], eyp[:])
            nc.gpsimd.collective_compute(
                "AllReduce",
                mybir.AluOpType.add,
                replica_groups=[list(range(CORES))],
                ins=[bin_t.opt()],
                outs=[bout_t.opt()],
            )
            # Result load waits on the collective — issue it from the ACT
            # HWDGE so its stalled descriptors can't block the SP-issued
            # mt-load stream.
            eyf = eyf_p.tile([P, s * EB], BF16, tag=f"eyf{s}")
            nc.scalar.dma_start(eyf[:], bout_t[:])
            return eyf

        eyfs = {}
        for gi, group in enumerate(AR_GROUPS):
            eyps = []
            for g in group:
                _, eyp = emit_p1_block(g, mt0_sb if g == 0 else None)
                eyps.append(eyp)
            eyfs[gi] = emit_ar_group(eyps)
        for gi, group in enumerate(AR_GROUPS):
            for i, g in enumerate(group):
                emit_p2_block(g, eyfs[gi], i * EB)

        # ---- Finalize: out = det ⊙ ny_sb.T -------------------------------
        out_sb = sbig.tile([P, NS], F32, tag="out_sb")
        for i0 in range(0, NJ, 4):
            nch = min(4, NJ - i0)
            tr = ps_tr.tile([P, 512], F32, tag="tr")
            for c in range(nch):
                i = i0 + c
                nc.tensor.transpose(
                    tr[:, c * P:(c + 1) * P],
                    ny_sb[:, i * P:(i + 1) * P],
                    id32[:],
                )
            for c in range(nch):
                i = i0 + c
                nc.vector.tensor_scalar_mul(
                    out_sb[:, i * P:(i + 1) * P],
                    tr[:, c * P:(c + 1) * P],
                    det_sb[:, i:i + 1],
                )
        nc.sync.dma_start(
            out.rearrange("(i p) f -> p i f", p=P),
            out_sb[:].rearrange("p (i f) -> p i f", f=F))

    return nc


@functools.lru_cache(maxsize=2)
def _compiled(cfg_items):
    cfg = dict(cfg_items)
    nc = bacc.Bacc(
        "TRN2",
        target_bir_lowering=False,
        debug=False,
        num_devices=cfg["CORES"],
    )
    build_kernel(nc, cfg)
    nc.compile()
    return nc


def shard_inputs(x, weight, MT, D_v_diag, D_e_diag, cfg):
    """Host-side sharding + dtype prep. Returns in_maps for the 8 cores."""
    N, E, IN, F, CORES = cfg["N"], cfg["E"], cfg["IN"], cfg["F"], cfg["CORES"]
    NS = N // CORES
    bf = ml_dtypes.bfloat16
    w_b = np.ascontiguousarray(np.asarray(weight, dtype=np.float32)).astype(bf)
    x_f = np.asarray(x, dtype=np.float32)
    mt_f = np.asarray(MT, dtype=np.float32)
    dv = np.asarray(D_v_diag, dtype=np.float32)
    de = np.asarray(D_e_diag, dtype=np.float32)
    # [P, E/P] with chunk index on the free axis
    dvt = np.ascontiguousarray(dv.reshape(E // 128, 128).T)
    in_maps = []
    for c in range(CORES):
        sl = slice(c * NS, (c + 1) * NS)
        det = np.ascontiguousarray(
            (0.5 * de[sl]).reshape(NS // 128, 128).T)
        in_maps.append({
            "mt": np.ascontiguousarray(mt_f[:, sl]).astype(bf),
            "xs": np.ascontiguousarray(x_f[sl]).astype(bf),
            "w": w_b,
            "dvt": dvt,
            "det": det,
        })
    return in_maps


def _run(x, weight, MT, D_v_diag, D_e_diag, cfg=None, trace=False):
    cfg = cfg or FULL_CFG
    nc = _compiled(tuple(sorted(cfg.items())))
    in_maps = shard_inputs(x, weight, MT, D_v_diag, D_e_diag, cfg)
    res = run_bass_kernel_spmd(
        nc, in_maps, core_ids=list(range(cfg["CORES"])), trace=trace)
    NS = cfg["N"] // cfg["CORES"]
    out = np.concatenate(
        [np.asarray(res.results[c]["out"]) for c in range(cfg["CORES"])],
        axis=0,
    ).astype(np.float32)
    return out, res


def kernel(x, weight, MT, D_v_diag, D_e_diag):
    out, _ = _run(x, weight, MT, D_v_diag, D_e_diag)
    return out
